# revision 33
# baseline (speedup 1.0000x reference)
"""Trainium2 Bass kernel for nn_MAE_CalcLoss_Raw (masked MSE loss).

reference math:
    masked   = mean_b[ mean_{i,d} (outputs[b, mask_id[b,i], d]   - orig[b, mask_id[b,i], d])^2 ]
    unmasked = mean_b[ mean_{i,d} (outputs[b, unmask_id[b,i], d] - orig[b, unmask_id[b,i], d])^2 ]
    loss = masked + 0.1 * unmasked

Rewrite: gathering rows by index (with repeats) is a weighted sum over
referenced (b, s) rows.  With cnt_m[b,s] = #occurrences of s in
mask_id[b], cnt_u likewise:

    loss = sum_{b,s} w[b,s] * ||outputs[b,s,:] - orig[b,s,:]||^2
    w[b,s] = cnt_m[b,s]/(B*Nm*D) + ALPHA*cnt_u[b,s]/(B*Nu*D)

Only ~63% of rows are referenced (2048 draws with replacement from 2048
rows -> 1-1/e distinct), so instead of streaming both tensors in full
(HBM-bound at ~358 GB/s/core = ~187 us) the kernel gathers just the
referenced rows (~42 MB/core -> ~120 us floor).

The gather uses the InstDMAGatherAnt custom GPSIMD instruction.  Its
Q7 descriptor generation costs ~8.8 ns/index and is serialized on the
Pool engine, so per-row gathers (~10.4k rows x 2 tensors/core) would be
Pool-bound at ~185 us.  Two mitigations make it DMA-bound instead:

  1. Runs of consecutive referenced rows are decomposed exactly into
     windows of {8,4,2,1} rows (one descriptor per window;
     elem_step=512 < elem_size allows windows at arbitrary row offsets
     via a manually-built overlapping access pattern).  ~5.05k
     windows/tensor/core.
  2. Gathers alternate between 2 SWDGE queues, whose descriptor
     generation runs on different Q7 cpu pairs and overlaps.

Per chunk (<=1024 gathered rows, <=2 MB/tensor): gather x, gather y
(Pool), subtract in place (DVE), then square+per-row-accumulate ops
(3/4 on ACT, 1/4 on DVE) into a [128, 92] accumulator DMA'd out raw;
the host applies the per-row histogram weights in float64 (pad slots
are masked out by weight==0).  Data-parallel over B: 8 samples/core.

Further structure: a warm-up 16-index gather + early load_library pay
the one-time ~5us IRAM ucode load while the idx plane streams in; all
index slots are padded with row 0 (always valid) because runtime
-1-trimmed gathers were observed to corrupt SWDGE ring state mid-NEFF;
the last chunks taper in size to shorten the post-DMA compute tail.

Measured on HW (8 cores, traced): ~145-160 us vs 222 us for the
full-streaming baseline (which runs at the 358 GB/s/core HBM limit;
this kernel beats it by reading 0.37x the bytes at ~350 GB/s/core).

If a window class overflows its compiled capacity (margins are ~3
sigma above the max observed per-core counts), the kernel falls back
to the full-streaming variant which is always correct.
"""

import numpy as np

ALPHA = 0.1
B, S, D = 64, 2048, 512
NM, NU = 1536, 512
N_CORES = 8
BPC = B // N_CORES            # samples per core
R = BPC * S                   # rows per core = 16384

# --- gather-kernel geometry ---
# (window_rows, [chunk slot counts]); caps are max-observed-per-core + >5
# sigma margin (max seen: w1 2391, w2 1764, w4 875, w8 162).  Pool-heavy
# classes (many descriptors per byte) go first so the kernel tail is small
# and DMA-bound.  Chunk slot counts must be multiples of 16; the last chunk
# of each class is small to shorten the pipeline tail.
CLASSES = [
    (8, [128, 48]),                  # cap 176 (max seen 162)
    (4, [256, 256, 256, 128]),       # cap 896 (max seen 875)
    (2, [512, 512, 512, 256]),       # cap 1792 (max seen 1764)
    (1, [1024, 768, 416, 208]),      # cap 2416 (max seen 2391)
]
import os as _os
if _os.environ.get("K_CLASSES"):
    CLASSES = [
        (int(p.split(":")[0]), [int(x) for x in p.split(":")[1].split("/")])
        for p in _os.environ["K_CLASSES"].split(",")
    ]
def _cdiv(a, b):
    return -(-a // b)


# per-chunk racc columns = ceil(cs/128) * w
NCOL = sum(_cdiv(cs, 128) * w for w, csl in CLASSES for cs in csl)
IDXCOL = sum(cs // 16 for _, csl in CLASSES for cs in csl)
CAPS = {w: sum(csl) for w, csl in CLASSES}
ACT_FRAC = 0.75               # fraction of per-chunk accum columns on ACT

# --- streaming-kernel geometry (fallback) ---
GROUPS = 8                    # 128-row groups per tile
TILE_ROWS = GROUPS * 128      # 1024 rows per tile (2 MB per tensor)
N_TILES_FULL = R // TILE_ROWS          # 16

_CACHE: dict = {}


def _build_gather_nc():
    import concourse.bacc as bacc
    import concourse.bass as bass
    import concourse.tile as tile
    import concourse.mybir as mybir
    import bass_rust

    f32 = mybir.dt.float32
    i16 = mybir.dt.int16

    _NPAD = NCOL
    nq = int(_os.environ.get("K_NQ", "2"))
    nc = bacc.Bacc(
        "TRN2",
        target_bir_lowering=False,
        debug=False,
        enable_asserts=False,
        num_devices=N_CORES,
        num_swdge_queues=nq,
    )
    x_d = nc.dram_tensor("x", [R, D], f32, kind="ExternalInput").ap()
    y_d = nc.dram_tensor("y", [R, D], f32, kind="ExternalInput").ap()
    idx_d = nc.dram_tensor("idx", [128, IDXCOL], i16, kind="ExternalInput").ap()
    dzi_d = nc.dram_tensor("dzi", [128, 1], i16, kind="ExternalInput").ap()
    p_d = nc.dram_tensor(
        "racc_out", [128, _NPAD], f32, kind="ExternalOutput").ap()

    # Overlapping window views: row-stride 512 elems, window length w*512.
    def win_view(base, w):
        if w == 1:
            return base
        v = base.copy()
        v.ap = bass_rust.VecI64Pair([[D, R - w + 1], [1, w * D]])
        return v

    xv = {w: win_view(x_d, w) for w, _ in CLASSES}
    yv = {w: win_view(y_d, w) for w, _ in CLASSES}

    with tile.TileContext(nc) as tc:
        with (
            tc.tile_pool(name="io", bufs=int(_os.environ.get("K_BUFS", "6"))) as io,
            tc.tile_pool(name="acc", bufs=1) as acc,
        ):
            # Load the extended-inst library (dma_gather) early so its ~5us
            # IRAM load overlaps the idx-plane DMA.
            from concourse.library_config import mlp as _mlp
            nc.gpsimd.load_library(_mlp)

            # Warm-up: a 16-index gather dispatched first makes the Pool Q7
            # pay the ~6us extended-instruction IRAM library load while the
            # main idx plane is still streaming in.
            dzi_sb = acc.tile([128, 1], i16, tag="dzi")
            nc.sync.dma_start(dzi_sb[:], dzi_d[:])
            warm = acc.tile([128, 1, D], f32, tag="warm")
            nc.gpsimd.dma_gather(warm[:], x_d, dzi_sb[:], 16, 16, D)

            idx_sb = acc.tile([128, IDXCOL], i16, tag="idx")
            nc.sync.dma_start(idx_sb[:], idx_d[:])
            racc = acc.tile([128, _NPAD], f32, tag="racc")

            icol = 0
            rcol = 0
            gidx = 0
            for w, csl in CLASSES:
                for cs in csl:
                    ccols = _cdiv(cs, 128)     # tile columns
                    icols = cs // 16           # idx columns this chunk
                    xt = io.tile([128, ccols, w * D], f32, tag="x")
                    yt = io.tile([128, ccols, w * D], f32, tag="y")
                    ixap = idx_sb[:, icol:icol + icols]
                    step = None if w == 1 else D
                    nc.gpsimd.dma_gather(
                        xt[:], xv[w], ixap, cs, cs, w * D, elem_step=step,
                        queue_num=gidx % nq)
                    gidx += 1
                    nc.gpsimd.dma_gather(
                        yt[:], yv[w], ixap, cs, cs, w * D, elem_step=step,
                        queue_num=gidx % nq)
                    gidx += 1
                    nc.vector.tensor_sub(xt[:], xt[:], yt[:])
                    ncols = ccols * w          # racc columns this chunk
                    nact = round(ACT_FRAC * ncols)
                    for g in range(ncols):
                        c, r = divmod(g, w)
                        src = xt[:, c, r * D:(r + 1) * D]
                        col = racc[:, rcol + g:rcol + g + 1]
                        if g < nact:
                            nc.scalar.activation(
                                src, src,
                                mybir.ActivationFunctionType.Square,
                                accum_out=col)
                        else:
                            nc.vector.scalar_tensor_tensor(
                                out=src, in0=src, scalar=1.0, in1=src,
                                op0=mybir.AluOpType.mult,
                                op1=mybir.AluOpType.mult,
                                accum_out=col)
                    icol += icols
                    rcol += ncols

            nc.sync.dma_start(p_d[:], racc[:])

    nc.compile()
    return nc


def _build_stream_nc():
    import concourse.bacc as bacc
    import concourse.bass as bass
    import concourse.tile as tile
    import concourse.mybir as mybir

    f32 = mybir.dt.float32
    ncol = N_TILES_FULL * GROUPS
    nc = bacc.Bacc(
        "TRN2",
        target_bir_lowering=False,
        debug=False,
        enable_asserts=False,
        num_devices=N_CORES,
    )
    x_d = nc.dram_tensor("x", [R, D], f32, kind="ExternalInput").ap()
    y_d = nc.dram_tensor("y", [R, D], f32, kind="ExternalInput").ap()
    p_d = nc.dram_tensor("racc_out", [128, ncol], f32, kind="ExternalOutput").ap()

    with tile.TileContext(nc) as tc:
        with (
            tc.tile_pool(name="io", bufs=4) as io,
            tc.tile_pool(name="acc", bufs=1) as acc,
        ):
            racc = acc.tile([128, ncol], f32, tag="racc")

            HG = GROUPS // 2  # half-tile: 4 groups, 1 MB per tensor
            n_halves = 2 * N_TILES_FULL
            for h in range(n_halves):
                if h == n_halves - 1:
                    # final half-tile in single-group chunks: shortens the
                    # compute tail after the last DMA lands
                    for g in range(HG):
                        j = h * HG + g
                        xg = io.tile([128, 1, D], f32, tag="xf")
                        nc.sync.dma_start(
                            xg[:],
                            x_d[bass.ts(j, 128), :].rearrange(
                                "(g p) d -> p g d", g=1, p=128
                            ),
                        )
                        yg = io.tile([128, 1, D], f32, tag="yf")
                        nc.sync.dma_start(
                            yg[:],
                            y_d[bass.ts(j, 128), :].rearrange(
                                "(g p) d -> p g d", g=1, p=128
                            ),
                        )
                        nc.vector.tensor_sub(xg[:], xg[:], yg[:])
                        if g == HG - 1:
                            nc.vector.scalar_tensor_tensor(
                                out=xg[:, 0, :],
                                in0=xg[:, 0, :],
                                scalar=1.0,
                                in1=xg[:, 0, :],
                                op0=mybir.AluOpType.mult,
                                op1=mybir.AluOpType.mult,
                                accum_out=racc[:, j : j + 1],
                            )
                        else:
                            nc.scalar.activation(
                                xg[:, 0, :],
                                xg[:, 0, :],
                                mybir.ActivationFunctionType.Square,
                                accum_out=racc[:, j : j + 1],
                            )
                    continue
                xt = io.tile([128, HG, D], f32, tag="x")
                yt = io.tile([128, HG, D], f32, tag="y")
                nc.sync.dma_start(
                    xt[:],
                    x_d[bass.ts(h, HG * 128), :].rearrange(
                        "(g p) d -> p g d", g=HG, p=128
                    ),
                )
                nc.sync.dma_start(
                    yt[:],
                    y_d[bass.ts(h, HG * 128), :].rearrange(
                        "(g p) d -> p g d", g=HG, p=128
                    ),
                )
                # diff in place on DVE
                nc.vector.tensor_sub(xt[:], xt[:], yt[:])
                # square + per-row accumulate: 3 groups on ACT, 1 on DVE
                for g in range(HG):
                    j = h * HG + g
                    if g == HG - 1:
                        nc.vector.scalar_tensor_tensor(
                            out=xt[:, g, :],
                            in0=xt[:, g, :],
                            scalar=1.0,
                            in1=xt[:, g, :],
                            op0=mybir.AluOpType.mult,
                            op1=mybir.AluOpType.mult,
                            accum_out=racc[:, j : j + 1],
                        )
                    else:
                        nc.scalar.activation(
                            xt[:, g, :],
                            xt[:, g, :],
                            mybir.ActivationFunctionType.Square,
                            accum_out=racc[:, j : j + 1],
                        )

            nc.sync.dma_start(p_d[:], racc[:])

    nc.compile()
    return nc


def _get_nc(kind: str):
    if kind not in _CACHE:
        _CACHE[kind] = (
            _build_gather_nc() if kind == "gather" else _build_stream_nc()
        )
    return _CACHE[kind]


def _hists(mask_id, unmask_id):
    rows = np.arange(B)[:, None]
    cm = np.zeros((B, S), np.float64)
    np.add.at(cm, (rows, mask_id.astype(np.int64)), 1.0)
    cu = np.zeros((B, S), np.float64)
    np.add.at(cu, (rows, unmask_id.astype(np.int64)), 1.0)
    return cm, cu


def _decompose(ref_c):
    """Runs of consecutive referenced rows -> exact {8,4,2,1} window cover.
    Returns {w: list of start rows} or None if any class overflows CAPS."""
    d = np.diff(np.concatenate([[0], ref_c.astype(np.int8), [0]]))
    starts = np.nonzero(d == 1)[0]
    ends = np.nonzero(d == -1)[0]
    by_w = {w: [] for w, _ in CLASSES}
    for s, e in zip(starts, ends):
        pos, L = int(s), int(e - s)
        for w in sorted(by_w, reverse=True):
            q, L = divmod(L, w)
            for _ in range(q):
                by_w[w].append(pos)
                pos += w
    for w, _ in CLASSES:
        if len(by_w[w]) > CAPS[w]:
            if _os.environ.get("K_TRUNC"):   # dev: truncate instead of fallback
                by_w[w] = by_w[w][: CAPS[w]]
            else:
                return None
    return by_w


def _gather_maps(x, y, w_full):
    """Per-core input maps + weight matrices for the gather kernel.
    Returns None if any core's window classes overflow capacity."""
    maps, wmats = [], []
    for c in range(N_CORES):
        w_c = w_full[c * R:(c + 1) * R]
        by_w = _decompose(w_c > 0)
        if by_w is None:
            return None, None
        idx_blocks = []
        wm = np.zeros((128, NCOL), np.float64)
        rcol = 0
        for w, csl in CLASSES:
            # pad with row 0 (always-valid window, weight 0): every slot is
            # gathered, so num_idxs_reg == num_idxs holds and no slot ever
            # holds stale SBUF garbage
            n_w = len(by_w[w])
            arr_all = np.zeros(sum(csl), np.int64)
            arr_all[:n_w] = by_w[w]
            off = 0
            for cs in csl:
                arr = arr_all[off:off + cs]
                blk = arr.reshape(cs // 16, 16).T
                idx_blocks.append(np.tile(blk, (8, 1)).astype(np.int16))
                i = np.arange(cs)
                valid = (off + i) < n_w
                pp, cc = i % 128, i // 128
                for r in range(w):
                    col = rcol + cc * w + r
                    wm[pp[valid], col[valid]] = w_c[arr[valid] + r]
                rcol += _cdiv(cs, 128) * w
                off += cs
        maps.append({
            "x": x[c * R:(c + 1) * R],
            "y": y[c * R:(c + 1) * R],
            "idx": np.ascontiguousarray(np.concatenate(idx_blocks, axis=1)),
            "dzi": np.zeros((128, 1), np.int16),
        })
        wmats.append(wm)
    return maps, wmats


def _stream_maps(x, y, w_full):
    maps, wmats = [], []
    for c in range(N_CORES):
        w_c = w_full[c * R:(c + 1) * R]
        maps.append({"x": x[c * R:(c + 1) * R], "y": y[c * R:(c + 1) * R]})
        wmats.append(
            w_c.reshape(N_TILES_FULL, GROUPS, 128)
            .transpose(2, 0, 1)
            .reshape(128, N_TILES_FULL * GROUPS)
        )
    return maps, wmats


def _in_maps(outputs, orig_image, mask_id, unmask_id, force_stream: bool = False):
    cm, cu = _hists(np.asarray(mask_id), np.asarray(unmask_id))
    w = (cm / (B * NM * D) + ALPHA * cu / (B * NU * D)).reshape(B * S)  # f64

    x = np.ascontiguousarray(np.asarray(outputs, dtype=np.float32)).reshape(B * S, D)
    y = np.ascontiguousarray(np.asarray(orig_image, dtype=np.float32)).reshape(B * S, D)

    if not force_stream:
        maps, wmats = _gather_maps(x, y, w)
        if maps is not None:
            return maps, "gather", wmats
    maps, wmats = _stream_maps(x, y, w)
    return maps, "stream", wmats


def _run(inputs: dict, trace: bool = False, force_stream: bool = False, **kw):
    from concourse.bass_utils import run_bass_kernel_spmd

    maps, kind, wmats = _in_maps(**inputs, force_stream=force_stream)
    nc = _get_nc(kind)
    res = run_bass_kernel_spmd(nc, maps, list(range(N_CORES)), trace=trace, **kw)
    total = np.float64(0.0)
    for c in range(N_CORES):
        racc = np.asarray(res.results[c]["racc_out"], dtype=np.float64)
        wm = wmats[c]
        m = wm != 0
        total += (racc[m] * wm[m]).sum()
    return np.asarray(total, dtype=np.float32), res


def kernel(outputs, orig_image, mask_id, unmask_id):
    outputs = np.asarray(outputs)
    orig_image = np.asarray(orig_image)
    mask_id = np.asarray(mask_id)
    unmask_id = np.asarray(unmask_id)
    assert outputs.shape == (B, S, D), outputs.shape
    assert orig_image.shape == (B, S, D), orig_image.shape
    assert mask_id.shape == (B, NM), mask_id.shape
    assert unmask_id.shape == (B, NU), unmask_id.shape
    out, _ = _run(
        {
            "outputs": outputs,
            "orig_image": orig_image,
            "mask_id": mask_id,
            "unmask_id": unmask_id,
        }
    )
    return out


# revision 34
# speedup vs baseline: 1.0830x; 1.0830x over previous
"""Trainium2 Bass kernel for nn_MAE_CalcLoss_Raw (masked MSE loss).

reference math:
    masked   = mean_b[ mean_{i,d} (outputs[b, mask_id[b,i], d]   - orig[b, mask_id[b,i], d])^2 ]
    unmasked = mean_b[ mean_{i,d} (outputs[b, unmask_id[b,i], d] - orig[b, unmask_id[b,i], d])^2 ]
    loss = masked + 0.1 * unmasked

Rewrite: gathering rows by index (with repeats) is a weighted sum over
referenced (b, s) rows.  With cnt_m[b,s] = #occurrences of s in
mask_id[b], cnt_u likewise:

    loss = sum_{b,s} w[b,s] * ||outputs[b,s,:] - orig[b,s,:]||^2
    w[b,s] = cnt_m[b,s]/(B*Nm*D) + ALPHA*cnt_u[b,s]/(B*Nu*D)

Only ~63% of rows are referenced (2048 draws with replacement from 2048
rows -> 1-1/e distinct), so instead of streaming both tensors in full
(HBM-bound at ~358 GB/s/core = ~187 us) the kernel gathers just the
referenced rows (~42 MB/core -> ~120 us floor).

The gather uses the InstDMAGatherAnt custom GPSIMD instruction.  Its
Q7 descriptor generation costs ~8.8 ns/index and is serialized on the
Pool engine, so per-row gathers (~10.4k rows x 2 tensors/core) would be
Pool-bound at ~185 us.  Two mitigations make it DMA-bound instead:

  1. Runs of consecutive referenced rows are decomposed exactly into
     windows of {8,4,2,1} rows (one descriptor per window;
     elem_step=512 < elem_size allows windows at arbitrary row offsets
     via a manually-built overlapping access pattern).  ~5.05k
     windows/tensor/core.
  2. Gathers alternate between 2 SWDGE queues, whose descriptor
     generation runs on different Q7 cpu pairs and overlaps.

Per chunk (<=1024 gathered rows, <=2 MB/tensor): gather x, gather y
(Pool), subtract in place (DVE), then square+per-row-accumulate ops
(3/4 on ACT, 1/4 on DVE) into a [128, 92] accumulator DMA'd out raw;
the host applies the per-row histogram weights in float64 (pad slots
are masked out by weight==0).  Data-parallel over B: 8 samples/core.

Further structure: a warm-up 16-index gather + early load_library pay
the one-time ~5us IRAM ucode load while the idx plane streams in; all
index slots are padded with row 0 (always valid) because runtime
-1-trimmed gathers were observed to corrupt SWDGE ring state mid-NEFF;
the last chunks taper in size to shorten the post-DMA compute tail.

Measured on HW (8 cores, traced): 145-158 us (best 144.9) vs 222 us
for the full-streaming baseline (which runs at the 358 GB/s/core HBM
limit; this kernel reads 0.37x the bytes at ~350 GB/s/core).  The
remaining time is ~16 us fixed startup (entry barriers + ucode IRAM
load) and ~20 us tail (last-chunk compute + teardown); mid-section is
DMA/compute co-limited (DVE 76%, ACT 53% busy).

If a window class overflows its compiled capacity (margins are ~3
sigma above the max observed per-core counts), the kernel falls back
to the full-streaming variant which is always correct.
"""

import numpy as np

ALPHA = 0.1
B, S, D = 64, 2048, 512
NM, NU = 1536, 512
N_CORES = 8
BPC = B // N_CORES            # samples per core
R = BPC * S                   # rows per core = 16384

# --- gather-kernel geometry ---
# (window_rows, [chunk slot counts]); caps are max-observed-per-core + >5
# sigma margin (max seen: w1 2391, w2 1764, w4 875, w8 162).  Pool-heavy
# classes (many descriptors per byte) go first so the kernel tail is small
# and DMA-bound.  Chunk slot counts must be multiples of 16; the last chunk
# of each class is small to shorten the pipeline tail.
CLASSES = [
    (8, [128, 48]),                  # cap 176 (max seen 162)
    (4, [256, 256, 256, 128]),       # cap 896 (max seen 875)
    (2, [512, 512, 512, 256]),       # cap 1792 (max seen 1764)
    (1, [1024, 768, 416, 208]),      # cap 2416 (max seen 2391)
]
import os as _os
if _os.environ.get("K_CLASSES"):
    CLASSES = [
        (int(p.split(":")[0]), [int(x) for x in p.split(":")[1].split("/")])
        for p in _os.environ["K_CLASSES"].split(",")
    ]
def _cdiv(a, b):
    return -(-a // b)


# per-chunk racc columns = ceil(cs/128) * w
NCOL = sum(_cdiv(cs, 128) * w for w, csl in CLASSES for cs in csl)
IDXCOL = sum(cs // 16 for _, csl in CLASSES for cs in csl)
CAPS = {w: sum(csl) for w, csl in CLASSES}
ACT_FRAC = 0.75               # fraction of per-chunk accum columns on ACT

# --- streaming-kernel geometry (fallback) ---
GROUPS = 8                    # 128-row groups per tile
TILE_ROWS = GROUPS * 128      # 1024 rows per tile (2 MB per tensor)
N_TILES_FULL = R // TILE_ROWS          # 16

_CACHE: dict = {}


def _build_gather_nc():
    import concourse.bacc as bacc
    import concourse.bass as bass
    import concourse.tile as tile
    import concourse.mybir as mybir
    import bass_rust

    f32 = mybir.dt.float32
    i16 = mybir.dt.int16

    _NPAD = NCOL
    nq = int(_os.environ.get("K_NQ", "2"))
    nc = bacc.Bacc(
        "TRN2",
        target_bir_lowering=False,
        debug=False,
        enable_asserts=False,
        num_devices=N_CORES,
        num_swdge_queues=nq,
    )
    x_d = nc.dram_tensor("x", [R, D], f32, kind="ExternalInput").ap()
    y_d = nc.dram_tensor("y", [R, D], f32, kind="ExternalInput").ap()
    idx_d = nc.dram_tensor("idx", [128, IDXCOL], i16, kind="ExternalInput").ap()
    dzi_d = nc.dram_tensor("dzi", [128, 1], i16, kind="ExternalInput").ap()
    p_d = nc.dram_tensor(
        "racc_out", [128, _NPAD], f32, kind="ExternalOutput").ap()

    # Overlapping window views: row-stride 512 elems, window length w*512.
    def win_view(base, w):
        if w == 1:
            return base
        v = base.copy()
        v.ap = bass_rust.VecI64Pair([[D, R - w + 1], [1, w * D]])
        return v

    xv = {w: win_view(x_d, w) for w, _ in CLASSES}
    yv = {w: win_view(y_d, w) for w, _ in CLASSES}

    with tile.TileContext(nc) as tc:
        with (
            tc.tile_pool(name="io", bufs=int(_os.environ.get("K_BUFS", "6"))) as io,
            tc.tile_pool(name="acc", bufs=1) as acc,
        ):
            # Load the extended-inst library (dma_gather) early so its ~5us
            # IRAM load overlaps the idx-plane DMA.
            from concourse.library_config import mlp as _mlp
            nc.gpsimd.load_library(_mlp)

            # Warm-up: a 16-index gather dispatched first makes the Pool Q7
            # pay the ~6us extended-instruction IRAM library load while the
            # main idx plane is still streaming in.
            dzi_sb = acc.tile([128, 1], i16, tag="dzi")
            nc.sync.dma_start(dzi_sb[:], dzi_d[:])
            warm = acc.tile([128, 1, D], f32, tag="warm")
            nc.gpsimd.dma_gather(warm[:], x_d, dzi_sb[:], 16, 16, D)

            idx_sb = acc.tile([128, IDXCOL], i16, tag="idx")
            nc.sync.dma_start(idx_sb[:], idx_d[:])
            racc = acc.tile([128, _NPAD], f32, tag="racc")

            icol = 0
            rcol = 0
            gidx = 0
            for w, csl in CLASSES:
                for cs in csl:
                    ccols = _cdiv(cs, 128)     # tile columns
                    icols = cs // 16           # idx columns this chunk
                    xt = io.tile([128, ccols, w * D], f32, tag="x")
                    yt = io.tile([128, ccols, w * D], f32, tag="y")
                    ixap = idx_sb[:, icol:icol + icols]
                    step = None if w == 1 else D
                    nc.gpsimd.dma_gather(
                        xt[:], xv[w], ixap, cs, cs, w * D, elem_step=step,
                        queue_num=gidx % nq)
                    gidx += 1
                    nc.gpsimd.dma_gather(
                        yt[:], yv[w], ixap, cs, cs, w * D, elem_step=step,
                        queue_num=gidx % nq)
                    gidx += 1
                    nc.vector.tensor_sub(xt[:], xt[:], yt[:])
                    ncols = ccols * w          # racc columns this chunk
                    nact = round(ACT_FRAC * ncols)
                    for g in range(ncols):
                        c, r = divmod(g, w)
                        src = xt[:, c, r * D:(r + 1) * D]
                        col = racc[:, rcol + g:rcol + g + 1]
                        if g < nact:
                            nc.scalar.activation(
                                src, src,
                                mybir.ActivationFunctionType.Square,
                                accum_out=col)
                        else:
                            nc.vector.scalar_tensor_tensor(
                                out=src, in0=src, scalar=1.0, in1=src,
                                op0=mybir.AluOpType.mult,
                                op1=mybir.AluOpType.mult,
                                accum_out=col)
                    icol += icols
                    rcol += ncols

            nc.sync.dma_start(p_d[:], racc[:])

    nc.compile()
    return nc


def _build_stream_nc():
    import concourse.bacc as bacc
    import concourse.bass as bass
    import concourse.tile as tile
    import concourse.mybir as mybir

    f32 = mybir.dt.float32
    ncol = N_TILES_FULL * GROUPS
    nc = bacc.Bacc(
        "TRN2",
        target_bir_lowering=False,
        debug=False,
        enable_asserts=False,
        num_devices=N_CORES,
    )
    x_d = nc.dram_tensor("x", [R, D], f32, kind="ExternalInput").ap()
    y_d = nc.dram_tensor("y", [R, D], f32, kind="ExternalInput").ap()
    p_d = nc.dram_tensor("racc_out", [128, ncol], f32, kind="ExternalOutput").ap()

    with tile.TileContext(nc) as tc:
        with (
            tc.tile_pool(name="io", bufs=4) as io,
            tc.tile_pool(name="acc", bufs=1) as acc,
        ):
            racc = acc.tile([128, ncol], f32, tag="racc")

            HG = GROUPS // 2  # half-tile: 4 groups, 1 MB per tensor
            n_halves = 2 * N_TILES_FULL
            for h in range(n_halves):
                if h == n_halves - 1:
                    # final half-tile in single-group chunks: shortens the
                    # compute tail after the last DMA lands
                    for g in range(HG):
                        j = h * HG + g
                        xg = io.tile([128, 1, D], f32, tag="xf")
                        nc.sync.dma_start(
                            xg[:],
                            x_d[bass.ts(j, 128), :].rearrange(
                                "(g p) d -> p g d", g=1, p=128
                            ),
                        )
                        yg = io.tile([128, 1, D], f32, tag="yf")
                        nc.sync.dma_start(
                            yg[:],
                            y_d[bass.ts(j, 128), :].rearrange(
                                "(g p) d -> p g d", g=1, p=128
                            ),
                        )
                        nc.vector.tensor_sub(xg[:], xg[:], yg[:])
                        if g == HG - 1:
                            nc.vector.scalar_tensor_tensor(
                                out=xg[:, 0, :],
                                in0=xg[:, 0, :],
                                scalar=1.0,
                                in1=xg[:, 0, :],
                                op0=mybir.AluOpType.mult,
                                op1=mybir.AluOpType.mult,
                                accum_out=racc[:, j : j + 1],
                            )
                        else:
                            nc.scalar.activation(
                                xg[:, 0, :],
                                xg[:, 0, :],
                                mybir.ActivationFunctionType.Square,
                                accum_out=racc[:, j : j + 1],
                            )
                    continue
                xt = io.tile([128, HG, D], f32, tag="x")
                yt = io.tile([128, HG, D], f32, tag="y")
                nc.sync.dma_start(
                    xt[:],
                    x_d[bass.ts(h, HG * 128), :].rearrange(
                        "(g p) d -> p g d", g=HG, p=128
                    ),
                )
                nc.sync.dma_start(
                    yt[:],
                    y_d[bass.ts(h, HG * 128), :].rearrange(
                        "(g p) d -> p g d", g=HG, p=128
                    ),
                )
                # diff in place on DVE
                nc.vector.tensor_sub(xt[:], xt[:], yt[:])
                # square + per-row accumulate: 3 groups on ACT, 1 on DVE
                for g in range(HG):
                    j = h * HG + g
                    if g == HG - 1:
                        nc.vector.scalar_tensor_tensor(
                            out=xt[:, g, :],
                            in0=xt[:, g, :],
                            scalar=1.0,
                            in1=xt[:, g, :],
                            op0=mybir.AluOpType.mult,
                            op1=mybir.AluOpType.mult,
                            accum_out=racc[:, j : j + 1],
                        )
                    else:
                        nc.scalar.activation(
                            xt[:, g, :],
                            xt[:, g, :],
                            mybir.ActivationFunctionType.Square,
                            accum_out=racc[:, j : j + 1],
                        )

            nc.sync.dma_start(p_d[:], racc[:])

    nc.compile()
    return nc


def _get_nc(kind: str):
    if kind not in _CACHE:
        _CACHE[kind] = (
            _build_gather_nc() if kind == "gather" else _build_stream_nc()
        )
    return _CACHE[kind]


def _hists(mask_id, unmask_id):
    rows = np.arange(B)[:, None]
    cm = np.zeros((B, S), np.float64)
    np.add.at(cm, (rows, mask_id.astype(np.int64)), 1.0)
    cu = np.zeros((B, S), np.float64)
    np.add.at(cu, (rows, unmask_id.astype(np.int64)), 1.0)
    return cm, cu


def _decompose(ref_c):
    """Runs of consecutive referenced rows -> exact {8,4,2,1} window cover.
    Returns {w: list of start rows} or None if any class overflows CAPS."""
    d = np.diff(np.concatenate([[0], ref_c.astype(np.int8), [0]]))
    starts = np.nonzero(d == 1)[0]
    ends = np.nonzero(d == -1)[0]
    by_w = {w: [] for w, _ in CLASSES}
    for s, e in zip(starts, ends):
        pos, L = int(s), int(e - s)
        for w in sorted(by_w, reverse=True):
            q, L = divmod(L, w)
            for _ in range(q):
                by_w[w].append(pos)
                pos += w
    for w, _ in CLASSES:
        if len(by_w[w]) > CAPS[w]:
            if _os.environ.get("K_TRUNC"):   # dev: truncate instead of fallback
                by_w[w] = by_w[w][: CAPS[w]]
            else:
                return None
    return by_w


def _gather_maps(x, y, w_full):
    """Per-core input maps + weight matrices for the gather kernel.
    Returns None if any core's window classes overflow capacity."""
    maps, wmats = [], []
    for c in range(N_CORES):
        w_c = w_full[c * R:(c + 1) * R]
        by_w = _decompose(w_c > 0)
        if by_w is None:
            return None, None
        idx_blocks = []
        wm = np.zeros((128, NCOL), np.float64)
        rcol = 0
        for w, csl in CLASSES:
            # pad with row 0 (always-valid window, weight 0): every slot is
            # gathered, so num_idxs_reg == num_idxs holds and no slot ever
            # holds stale SBUF garbage
            n_w = len(by_w[w])
            arr_all = np.zeros(sum(csl), np.int64)
            arr_all[:n_w] = by_w[w]
            off = 0
            for cs in csl:
                arr = arr_all[off:off + cs]
                blk = arr.reshape(cs // 16, 16).T
                idx_blocks.append(np.tile(blk, (8, 1)).astype(np.int16))
                i = np.arange(cs)
                valid = (off + i) < n_w
                pp, cc = i % 128, i // 128
                for r in range(w):
                    col = rcol + cc * w + r
                    wm[pp[valid], col[valid]] = w_c[arr[valid] + r]
                rcol += _cdiv(cs, 128) * w
                off += cs
        maps.append({
            "x": x[c * R:(c + 1) * R],
            "y": y[c * R:(c + 1) * R],
            "idx": np.ascontiguousarray(np.concatenate(idx_blocks, axis=1)),
            "dzi": np.zeros((128, 1), np.int16),
        })
        wmats.append(wm)
    return maps, wmats


def _stream_maps(x, y, w_full):
    maps, wmats = [], []
    for c in range(N_CORES):
        w_c = w_full[c * R:(c + 1) * R]
        maps.append({"x": x[c * R:(c + 1) * R], "y": y[c * R:(c + 1) * R]})
        wmats.append(
            w_c.reshape(N_TILES_FULL, GROUPS, 128)
            .transpose(2, 0, 1)
            .reshape(128, N_TILES_FULL * GROUPS)
        )
    return maps, wmats


def _in_maps(outputs, orig_image, mask_id, unmask_id, force_stream: bool = False):
    cm, cu = _hists(np.asarray(mask_id), np.asarray(unmask_id))
    w = (cm / (B * NM * D) + ALPHA * cu / (B * NU * D)).reshape(B * S)  # f64

    x = np.ascontiguousarray(np.asarray(outputs, dtype=np.float32)).reshape(B * S, D)
    y = np.ascontiguousarray(np.asarray(orig_image, dtype=np.float32)).reshape(B * S, D)

    if not force_stream:
        maps, wmats = _gather_maps(x, y, w)
        if maps is not None:
            return maps, "gather", wmats
    maps, wmats = _stream_maps(x, y, w)
    return maps, "stream", wmats


def _run(inputs: dict, trace: bool = False, force_stream: bool = False, **kw):
    from concourse.bass_utils import run_bass_kernel_spmd

    maps, kind, wmats = _in_maps(**inputs, force_stream=force_stream)
    nc = _get_nc(kind)
    res = run_bass_kernel_spmd(nc, maps, list(range(N_CORES)), trace=trace, **kw)
    total = np.float64(0.0)
    for c in range(N_CORES):
        racc = np.asarray(res.results[c]["racc_out"], dtype=np.float64)
        wm = wmats[c]
        m = wm != 0
        total += (racc[m] * wm[m]).sum()
    return np.asarray(total, dtype=np.float32), res


def kernel(outputs, orig_image, mask_id, unmask_id):
    outputs = np.asarray(outputs)
    orig_image = np.asarray(orig_image)
    mask_id = np.asarray(mask_id)
    unmask_id = np.asarray(unmask_id)
    assert outputs.shape == (B, S, D), outputs.shape
    assert orig_image.shape == (B, S, D), orig_image.shape
    assert mask_id.shape == (B, NM), mask_id.shape
    assert unmask_id.shape == (B, NU), unmask_id.shape
    out, _ = _run(
        {
            "outputs": outputs,
            "orig_image": orig_image,
            "mask_id": mask_id,
            "unmask_id": unmask_id,
        }
    )
    return out
